# revision 3
# baseline (speedup 1.0000x reference)
"""BottomPool (cumulative max along H) Trainium2 Bass kernel.

Full input x: (16, 256, 128, 128) fp32. out[b,c,h,w] = max_{h'<=h} x[b,c,h',w].

Strategy: data-parallel over the 4096 (b,c) planes -> 512 planes per core;
device I/O in fp16 (host fp32<->fp16 conversion; rounding is monotone so
cummax commutes with it; max rel err 2^-11 << the 2e-2 gate).

The kernel is DMA-bound. Three trace-driven facts shape the schedule:
 1. The 16 SDMA engines sustain ~26.8 GB/s each (~429 GB/s/core). DMA
    descriptors are assigned round-robin engine = desc_index % 16 per
    dma_start (descs ordered by partition), verified by probe.
 2. SDMA engine 15 intermittently degrades to ~78% throughput (known
    trn2 erratum, engines 7/15). With a zero-slack schedule its backlog
    serializes at the end (+20us). Fix: per-column-split layout so every
    partition carries 400 "A" columns (uniform transfers, all engines)
    and the 120 partitions p with p%16!=15 carry 120 extra "B" columns,
    loaded/stored via 15-partition transfers whose descriptors land on
    engines 0-14 only. Engine 15 then gets ~77% of the per-engine bytes:
    balanced when it is degraded, harmless when healthy.
 3. Loads must own the bus first: all of SBUF input is resident (one
    [128, 128, 520] tile, 130KB/partition), the serial DVE cummax chain
    runs IN PLACE (row = max(row, prev_row), one [128,520] tensor_max
    per row), and stores queue FIFO behind loads on the same HWDGE ring
    (nc.sync), each gated only by its rows' chain semaphore.

The B columns sit at row offsets 400:520 so one DVE op per row covers
A+B. Partitions p%16==15 hold garbage in B columns (computed, never
stored). B transfers are split into two h-halves so the chain's first
rows don't wait on full-H B loads.
"""

import numpy as np

import concourse.tile as tile
from concourse import bacc, mybir
from concourse.bass_utils import run_bass_kernel_spmd

N_CORES = 8
B, C, H, W = 16, 256, 128, 128
P = 128  # SBUF partitions
PLANES_PER_CORE = (B * C) // N_CORES  # 512
NCOLS = PLANES_PER_CORE * W  # 65536 independent cummax columns per core
QA = 400  # A-block columns per partition (all 128 partitions)
QB = 120  # B-block columns per partition (120 partitions, p%16 != 15)
QW2 = QA + QB  # SBUF row length
NB_P = 120  # B-block partitions
NA = P * QA  # 51200 A columns
NB = NB_P * QB  # 14400 B slots (>= 14336 used; rest zero-pad)
HH = H // 2  # B transfers come in two h-halves
HS = 16  # A-block h-tile rows
DTYPE = "float16"
NP_DTYPE = np.float16


def build_module(n_cores=N_CORES):
    """Per-core Bass module (same program on all cores).

    I/O (host-packed, see make_in_maps):
      xa/ya: [128, 128, QA]         one cummax column per (partition, j)
      xb/yb: [120, 2, 64, QB]       h-halves; dram partition 15k+i maps to
                                    SBUF partition 16k+i (skips p%16==15)
    """
    mdt = getattr(mybir.dt, DTYPE)
    nc = bacc.Bacc(
        "TRN2", target_bir_lowering=False, debug=False, num_devices=n_cores
    )
    xa = nc.dram_tensor("xa", [P, H, QA], mdt, kind="ExternalInput").ap()
    xb = nc.dram_tensor("xb", [NB_P, 2, HH, QB], mdt,
                        kind="ExternalInput").ap()
    ya = nc.dram_tensor("ya", [P, H, QA], mdt, kind="ExternalOutput").ap()
    yb = nc.dram_tensor("yb", [NB_P, 2, HH, QB], mdt,
                        kind="ExternalOutput").ap()

    with tile.TileContext(nc) as tc:
        with tc.tile_pool(name="pin", bufs=1) as pin:
            t = pin.tile([P, H, QW2], mdt)
            # --- loads, all on the SP ring (FIFO): B-h0, A0, B-h1, A1..A7
            for k in range(8):
                nc.sync.dma_start(
                    t[16 * k:16 * k + 15, 0:HH, QA:QW2],
                    xb[15 * k:15 * k + 15, 0],
                )
            nc.sync.dma_start(t[:, 0:HS, 0:QA], xa[:, 0:HS, :])
            for k in range(8):
                nc.sync.dma_start(
                    t[16 * k:16 * k + 15, HH:H, QA:QW2],
                    xb[15 * k:15 * k + 15, 1],
                )
            for ti in range(1, H // HS):
                nc.sync.dma_start(
                    t[:, ti * HS:(ti + 1) * HS, 0:QA],
                    xa[:, ti * HS:(ti + 1) * HS, :],
                )
            # --- serial cummax chain, in place (row 0 is identity)
            prev = t[:, 0, :]
            for h in range(1, H):
                cur = t[:, h, :]
                nc.vector.tensor_max(cur, cur, prev)
                prev = cur
            # --- stores, same ring: A0-A3, B-h0, A4-A7, B-h1
            for ti in range(4):
                nc.sync.dma_start(
                    ya[:, ti * HS:(ti + 1) * HS, :],
                    t[:, ti * HS:(ti + 1) * HS, 0:QA],
                )
            for k in range(8):
                nc.sync.dma_start(
                    yb[15 * k:15 * k + 15, 0],
                    t[16 * k:16 * k + 15, 0:HH, QA:QW2],
                )
            for ti in range(4, H // HS):
                nc.sync.dma_start(
                    ya[:, ti * HS:(ti + 1) * HS, :],
                    t[:, ti * HS:(ti + 1) * HS, 0:QA],
                )
            for k in range(8):
                nc.sync.dma_start(
                    yb[15 * k:15 * k + 15, 1],
                    t[16 * k:16 * k + 15, HH:H, QA:QW2],
                )
    nc.compile()
    return nc


_NC_CACHE = {}


def _get_module():
    if "nc" not in _NC_CACHE:
        _NC_CACHE["nc"] = build_module()
    return _NC_CACHE["nc"]


def make_in_maps(x: np.ndarray) -> list:
    """fp32 (B,C,H,W) -> per-core {xa: [128,H,400], xb: [120,2,64,120]}.

    Column c = plane*W + w (plane local to the core). A-block: partition p
    holds columns [p*QA, (p+1)*QA). B-block: columns NA..NCOLS-1 (+ pad)
    laid out [120, QB] by dram-partition-major.
    """
    flat = np.asarray(x).reshape(B * C, H, W).astype(NP_DTYPE)
    maps = []
    for k in range(N_CORES):
        blk = flat[k * PLANES_PER_CORE:(k + 1) * PLANES_PER_CORE]
        m = np.ascontiguousarray(blk.transpose(0, 2, 1)).reshape(NCOLS, H)
        a = np.ascontiguousarray(
            m[:NA].reshape(P, QA, H).transpose(0, 2, 1)
        )
        bcols = np.concatenate(
            [m[NA:], np.zeros((NB - (NCOLS - NA), H), NP_DTYPE)], axis=0
        )
        b = np.ascontiguousarray(
            bcols.reshape(NB_P, QB, H).transpose(0, 2, 1)
        ).reshape(NB_P, 2, HH, QB)
        maps.append({"xa": a, "xb": b})
    return maps


def assemble_out(results) -> np.ndarray:
    """Per-core {ya, yb} -> fp32 (B,C,H,W)."""
    blocks = []
    for r in results:
        ma = r["ya"].transpose(0, 2, 1).reshape(NA, H)
        mb = (
            r["yb"].reshape(NB_P, H, QB).transpose(0, 2, 1).reshape(NB, H)
        )[: NCOLS - NA]
        m = np.concatenate([ma, mb], axis=0)  # [NCOLS, H]
        blk = m.reshape(PLANES_PER_CORE, W, H).transpose(0, 2, 1)
        blocks.append(blk)
    out = np.concatenate(blocks, axis=0)
    return out.reshape(B, C, H, W).astype(np.float32)


def kernel(x: np.ndarray) -> np.ndarray:
    assert x.shape == (B, C, H, W), x.shape
    in_maps = make_in_maps(x)
    nc = _get_module()
    res = run_bass_kernel_spmd(nc, in_maps, list(range(N_CORES)))
    return assemble_out(res.results)


# revision 6
# speedup vs baseline: 1.4876x; 1.4876x over previous
"""BottomPool (cumulative max along H) Trainium2 Bass kernel.

Full input x: (16, 256, 128, 128) fp32. out[b,c,h,w] = max_{h'<=h} x[b,c,h',w].

Strategy: data-parallel over the 4096 (b,c) planes -> 512 planes per core;
device I/O in fp16 (host fp32<->fp16 conversion; rounding is monotone so
cummax commutes with it; max rel err 2^-11 << the 2e-2 gate).

The kernel is DMA-bound. Three trace-driven facts shape the schedule:
 1. The 16 SDMA engines sustain ~26.8 GB/s each (~429 GB/s/core). DMA
    descriptors are assigned round-robin engine = desc_index % 16 per
    dma_start (descs ordered by partition), verified by probe.
 2. SDMA engine 15 intermittently degrades to ~78% throughput (known
    trn2 erratum, engines 7/15). With a zero-slack schedule its backlog
    serializes at the end (+20us). Fix: per-column-split layout so every
    partition carries 400 "A" columns (uniform transfers, all engines)
    and the 120 partitions p with p%16!=15 carry 120 extra "B" columns,
    loaded/stored via 15-partition transfers whose descriptors land on
    engines 0-14 only. Engine 15 then gets ~77% of the per-engine bytes:
    balanced when it is degraded, harmless when healthy.
 3. Loads must own the bus first: all of SBUF input is resident (one
    [128, 128, 520] tile, 130KB/partition), the serial DVE cummax chain
    runs IN PLACE (row = max(row, prev_row), one [128,520] tensor_max
    per row), and stores queue FIFO behind loads on the same HWDGE ring
    (nc.sync), each gated only by its rows' chain semaphore.

The B columns sit at row offsets 400:520 so one DVE op per row covers
A+B. Partitions p%16==15 hold garbage in B columns (computed, never
stored). B transfers are split into two h-halves so the chain's first
rows don't wait on full-H B loads.
"""

import numpy as np

import concourse.tile as tile
from concourse import bacc, mybir
from concourse.bass_utils import run_bass_kernel_spmd

N_CORES = 8
B, C, H, W = 16, 256, 128, 128
P = 128  # SBUF partitions
PLANES_PER_CORE = (B * C) // N_CORES  # 512
NCOLS = PLANES_PER_CORE * W  # 65536 independent cummax columns per core
QA = 400  # A-block columns per partition (all 128 partitions)
QB = 120  # B-block columns per partition (120 partitions, p%16 != 15)
QW2 = QA + QB  # SBUF row length
NB_P = 120  # B-block partitions
NA = P * QA  # 51200 A columns
NB = NB_P * QB  # 14400 B slots (>= 14336 used; rest zero-pad)
HH = H // 2  # B transfers come in two h-halves
HS = 16  # A-block h-tile rows
DTYPE = "float16"
NP_DTYPE = np.float16


def build_module(n_cores=N_CORES):
    """Per-core Bass module (same program on all cores).

    I/O (host-packed, see make_in_maps):
      xa/ya: [128, 128, QA]         one cummax column per (partition, j)
      xb/yb: [120, 2, 64, QB]       h-halves; dram partition 15k+i maps to
                                    SBUF partition 16k+i (skips p%16==15)
    """
    mdt = getattr(mybir.dt, DTYPE)
    nc = bacc.Bacc(
        "TRN2", target_bir_lowering=False, debug=False, num_devices=n_cores
    )
    xa = nc.dram_tensor("xa", [P, H, QA], mdt, kind="ExternalInput").ap()
    xb = nc.dram_tensor("xb", [NB_P, 2, HH, QB], mdt,
                        kind="ExternalInput").ap()
    ya = nc.dram_tensor("ya", [P, H, QA], mdt, kind="ExternalOutput").ap()
    yb = nc.dram_tensor("yb", [NB_P, 2, HH, QB], mdt,
                        kind="ExternalOutput").ap()

    NBLK = H // 8  # 16 blocks of 8 rows; row h = (blk, s) = (h//8, h%8)
    with tile.TileContext(nc) as tc:
        with (
            tc.tile_pool(name="pa", bufs=1) as pa,
            tc.tile_pool(name="pb", bufs=1) as pb,
        ):
            # Separate tiles so every DMA is per-partition contiguous
            # (16KB-class descriptors; a fused 520-wide row produced 240B
            # descriptor chunks and collapsed to ~200 GB/s). tb is 4D so
            # the blocked-scan passes are plain strided slices.
            ta = pa.tile([P, H, QA], mdt)
            tb = pb.tile([P, NBLK, 8, QB], mdt)
            # --- loads, all on the SP ring (FIFO): B-h0, A0, B-h1, A1..A7
            for k in range(8):
                nc.sync.dma_start(
                    tb[16 * k:16 * k + 15, 0:NBLK // 2, :, :],
                    xb[15 * k:15 * k + 15, 0],
                )
            nc.sync.dma_start(ta[:, 0:HS, :], xa[:, 0:HS, :])
            for k in range(8):
                nc.sync.dma_start(
                    tb[16 * k:16 * k + 15, NBLK // 2:NBLK, :, :],
                    xb[15 * k:15 * k + 15, 1],
                )
            for ti in range(1, H // HS):
                nc.sync.dma_start(
                    ta[:, ti * HS:(ti + 1) * HS, :],
                    xa[:, ti * HS:(ti + 1) * HS, :],
                )

            # --- B blocked cummax (all on DVE, in place), per h-half:
            # pass1: local scan inside each 8-row block (7 strided ops)
            # chain: serialize block-final rows (7-8 small ops)
            # pass3: apply carry C_{blk-1} to rows 0..6 of each block
            def b_passes(b0, b1):
                for s in range(1, 8):
                    nc.vector.tensor_max(
                        tb[:, b0:b1, s, :],
                        tb[:, b0:b1, s, :],
                        tb[:, b0:b1, s - 1, :],
                    )
                for blk in range(max(b0, 1), b1):
                    nc.vector.tensor_max(
                        tb[:, blk, 7, :],
                        tb[:, blk, 7, :],
                        tb[:, blk - 1, 7, :],
                    )
                c0 = max(b0, 1)
                for s in range(0, 7):
                    nc.vector.tensor_max(
                        tb[:, c0:b1, s, :],
                        tb[:, c0:b1, s, :],
                        tb[:, c0 - 1:b1 - 1, 7, :],
                    )

            b_passes(0, NBLK // 2)
            # --- A serial cummax chain, in place (row 0 is identity)
            prev_a = ta[:, 0, :]
            for h in range(1, H):
                cur_a = ta[:, h, :]
                nc.vector.tensor_max(cur_a, cur_a, prev_a)
                prev_a = cur_a
            b_passes(NBLK // 2, NBLK)
            # --- stores, same ring: A0-A3, B-h0, A4-A7, B-h1
            for ti in range(4):
                nc.sync.dma_start(
                    ya[:, ti * HS:(ti + 1) * HS, :],
                    ta[:, ti * HS:(ti + 1) * HS, :],
                )
            for k in range(8):
                nc.sync.dma_start(
                    yb[15 * k:15 * k + 15, 0],
                    tb[16 * k:16 * k + 15, 0:NBLK // 2, :, :],
                )
            for ti in range(4, H // HS):
                nc.sync.dma_start(
                    ya[:, ti * HS:(ti + 1) * HS, :],
                    ta[:, ti * HS:(ti + 1) * HS, :],
                )
            for k in range(8):
                nc.sync.dma_start(
                    yb[15 * k:15 * k + 15, 1],
                    tb[16 * k:16 * k + 15, NBLK // 2:NBLK, :, :],
                )
    nc.compile()
    return nc


_NC_CACHE = {}


def _get_module():
    if "nc" not in _NC_CACHE:
        _NC_CACHE["nc"] = build_module()
    return _NC_CACHE["nc"]


def make_in_maps(x: np.ndarray) -> list:
    """fp32 (B,C,H,W) -> per-core {xa: [128,H,400], xb: [120,2,64,120]}.

    Column c = plane*W + w (plane local to the core). A-block: partition p
    holds columns [p*QA, (p+1)*QA). B-block: columns NA..NCOLS-1 (+ pad)
    laid out [120, QB] by dram-partition-major.
    """
    flat = np.asarray(x).reshape(B * C, H, W).astype(NP_DTYPE)
    maps = []
    for k in range(N_CORES):
        blk = flat[k * PLANES_PER_CORE:(k + 1) * PLANES_PER_CORE]
        m = np.ascontiguousarray(blk.transpose(0, 2, 1)).reshape(NCOLS, H)
        a = np.ascontiguousarray(
            m[:NA].reshape(P, QA, H).transpose(0, 2, 1)
        )
        bcols = np.concatenate(
            [m[NA:], np.zeros((NB - (NCOLS - NA), H), NP_DTYPE)], axis=0
        )
        b = np.ascontiguousarray(
            bcols.reshape(NB_P, QB, H).transpose(0, 2, 1)
        ).reshape(NB_P, 2, HH, QB)
        maps.append({"xa": a, "xb": b})
    return maps


def assemble_out(results) -> np.ndarray:
    """Per-core {ya, yb} -> fp32 (B,C,H,W)."""
    blocks = []
    for r in results:
        ma = r["ya"].transpose(0, 2, 1).reshape(NA, H)
        mb = (
            r["yb"].reshape(NB_P, H, QB).transpose(0, 2, 1).reshape(NB, H)
        )[: NCOLS - NA]
        m = np.concatenate([ma, mb], axis=0)  # [NCOLS, H]
        blk = m.reshape(PLANES_PER_CORE, W, H).transpose(0, 2, 1)
        blocks.append(blk)
    out = np.concatenate(blocks, axis=0)
    return out.reshape(B, C, H, W).astype(np.float32)


def kernel(x: np.ndarray) -> np.ndarray:
    assert x.shape == (B, C, H, W), x.shape
    in_maps = make_in_maps(x)
    nc = _get_module()
    res = run_bass_kernel_spmd(nc, in_maps, list(range(N_CORES)))
    return assemble_out(res.results)


# revision 9
# speedup vs baseline: 1.4976x; 1.0067x over previous
"""BottomPool (cumulative max along H) Trainium2 Bass kernel.

Full input x: (16, 256, 128, 128) fp32. out[b,c,h,w] = max_{h'<=h} x[b,c,h',w].

Strategy: data-parallel over the 4096 (b,c) planes -> 512 planes per core;
device I/O in fp16 (host fp32<->fp16 conversion; rounding is monotone so
cummax commutes with it; max rel err 2^-11 << the 2e-2 gate).

The kernel is DMA-bound. Three trace-driven facts shape the schedule:
 1. The 16 SDMA engines sustain ~26.8 GB/s each (~429 GB/s/core). DMA
    descriptors are assigned round-robin engine = desc_index % 16 per
    dma_start (descs ordered by partition), verified by probe.
 2. SDMA engine 15 intermittently degrades to ~78% throughput (known
    trn2 erratum, engines 7/15). With a zero-slack schedule its backlog
    serializes at the end (+20us). Fix: per-column-split layout so every
    partition carries 400 "A" columns (uniform transfers, all engines)
    and the 120 partitions p with p%16!=15 carry 120 extra "B" columns,
    loaded/stored via 15-partition transfers whose descriptors land on
    engines 0-14 only. Engine 15 then gets ~77% of the per-engine bytes:
    balanced when it is degraded, harmless when healthy.
 3. Loads must own the bus first: all of SBUF input is resident (one
    [128, 128, 520] tile, 130KB/partition), the serial DVE cummax chain
    runs IN PLACE (row = max(row, prev_row), one [128,520] tensor_max
    per row), and stores queue FIFO behind loads on the same HWDGE ring
    (nc.sync), each gated only by its rows' chain semaphore.

The B columns sit at row offsets 400:520 so one DVE op per row covers
A+B. Partitions p%16==15 hold garbage in B columns (computed, never
stored). B transfers are split into two h-halves so the chain's first
rows don't wait on full-H B loads.
"""

import numpy as np

import concourse.tile as tile
from concourse import bacc, mybir
from concourse.bass_utils import run_bass_kernel_spmd

N_CORES = 8
B, C, H, W = 16, 256, 128, 128
P = 128  # SBUF partitions
PLANES_PER_CORE = (B * C) // N_CORES  # 512
NCOLS = PLANES_PER_CORE * W  # 65536 independent cummax columns per core
QA = 400  # A-block columns per partition (all 128 partitions)
QB = 120  # B-block columns per partition (120 partitions, p%16 != 15)
QW2 = QA + QB  # SBUF row length
NB_P = 120  # B-block partitions
NA = P * QA  # 51200 A columns
NB = NB_P * QB  # 14400 B slots (>= 14336 used; rest zero-pad)
HH = H // 2  # B transfers come in two h-halves
HS = 16  # A-block h-tile rows
DTYPE = "float16"
NP_DTYPE = np.float16


def build_module(n_cores=N_CORES):
    """Per-core Bass module (same program on all cores).

    I/O (host-packed, see make_in_maps):
      xa/ya: [128, 128, QA]         one cummax column per (partition, j)
      xb/yb: [120, 2, 64, QB]       h-halves; dram partition 15k+i maps to
                                    SBUF partition 16k+i (skips p%16==15)
    """
    mdt = getattr(mybir.dt, DTYPE)
    nc = bacc.Bacc(
        "TRN2", target_bir_lowering=False, debug=False, num_devices=n_cores
    )
    xa = nc.dram_tensor("xa", [P, H, QA], mdt, kind="ExternalInput").ap()
    xb = nc.dram_tensor("xb", [NB_P, H // 8, 8, QB], mdt,
                        kind="ExternalInput").ap()
    ya = nc.dram_tensor("ya", [P, H, QA], mdt, kind="ExternalOutput").ap()
    yb = nc.dram_tensor("yb", [NB_P, H // 8, 8, QB], mdt,
                        kind="ExternalOutput").ap()

    NBLK = H // 8  # 16 blocks of 8 rows; row h = (blk, s) = (h//8, h%8)
    with tile.TileContext(nc) as tc:
        with (
            tc.tile_pool(name="pa", bufs=1) as pa,
            tc.tile_pool(name="pb", bufs=1) as pb,
        ):
            # Separate tiles so every DMA is per-partition contiguous
            # (16KB-class descriptors; a fused 520-wide row produced 240B
            # descriptor chunks and collapsed to ~200 GB/s). tb is 4D so
            # the blocked-scan passes are plain strided slices.
            ta = pa.tile([P, H, QA], mdt)
            tb = pb.tile([P, NBLK, 8, QB], mdt)
            # --- loads, all on the SP ring (FIFO). 16 transfers total so
            # the 8 recycled DMA semaphores never stall issue (v5's 25
            # loads serialized issue at ~2us each). Order: A0, B0-3, A1,
            # B4-7, A2..A7 -- the chain starts on A0 ASAP and A-loads
            # stay ahead of the chain.
            nc.sync.dma_start(ta[:, 0:HS, :], xa[:, 0:HS, :])
            for k in range(4):
                nc.sync.dma_start(
                    tb[16 * k:16 * k + 15, :, :, :],
                    xb[15 * k:15 * k + 15],
                )
            nc.sync.dma_start(ta[:, HS:2 * HS, :], xa[:, HS:2 * HS, :])
            for k in range(4, 8):
                nc.sync.dma_start(
                    tb[16 * k:16 * k + 15, :, :, :],
                    xb[15 * k:15 * k + 15],
                )
            for ti in range(2, H // HS):
                nc.sync.dma_start(
                    ta[:, ti * HS:(ti + 1) * HS, :],
                    xa[:, ti * HS:(ti + 1) * HS, :],
                )

            # --- A serial cummax chain first (it feeds 13.1MB of
            # A-stores, the bulk of the store stream), in place.
            prev_a = ta[:, 0, :]
            for h in range(1, H):
                cur_a = ta[:, h, :]
                nc.vector.tensor_max(cur_a, cur_a, prev_a)
                prev_a = cur_a

            # --- B blocked cummax (DVE, in place), per h-half:
            # pass1: local scan inside each 8-row block (7 strided ops)
            # chain: serialize block-final rows (7-8 small ops)
            # pass3: apply carry C_{blk-1} to rows 0..6 of each block
            def b_passes(b0, b1):
                for s in range(1, 8):
                    nc.vector.tensor_max(
                        tb[:, b0:b1, s, :],
                        tb[:, b0:b1, s, :],
                        tb[:, b0:b1, s - 1, :],
                    )
                for blk in range(max(b0, 1), b1):
                    nc.vector.tensor_max(
                        tb[:, blk, 7, :],
                        tb[:, blk, 7, :],
                        tb[:, blk - 1, 7, :],
                    )
                c0 = max(b0, 1)
                for s in range(0, 7):
                    nc.vector.tensor_max(
                        tb[:, c0:b1, s, :],
                        tb[:, c0:b1, s, :],
                        tb[:, c0 - 1:b1 - 1, 7, :],
                    )

            b_passes(0, NBLK // 2)
            b_passes(NBLK // 2, NBLK)
            # --- stores, same ring: A0..A7, B-h0 (8), B-h1 (8)
            for ti in range(H // HS):
                nc.sync.dma_start(
                    ya[:, ti * HS:(ti + 1) * HS, :],
                    ta[:, ti * HS:(ti + 1) * HS, :],
                )
            for k in range(8):
                nc.sync.dma_start(
                    yb[15 * k:15 * k + 15, 0:NBLK // 2],
                    tb[16 * k:16 * k + 15, 0:NBLK // 2, :, :],
                )
            for k in range(8):
                nc.sync.dma_start(
                    yb[15 * k:15 * k + 15, NBLK // 2:NBLK],
                    tb[16 * k:16 * k + 15, NBLK // 2:NBLK, :, :],
                )
    nc.compile()
    return nc


_NC_CACHE = {}


def _get_module():
    if "nc" not in _NC_CACHE:
        _NC_CACHE["nc"] = build_module()
    return _NC_CACHE["nc"]


def make_in_maps(x: np.ndarray) -> list:
    """fp32 (B,C,H,W) -> per-core {xa: [128,H,400], xb: [120,2,64,120]}.

    Column c = plane*W + w (plane local to the core). A-block: partition p
    holds columns [p*QA, (p+1)*QA). B-block: columns NA..NCOLS-1 (+ pad)
    laid out [120, QB] by dram-partition-major.
    """
    flat = np.asarray(x).reshape(B * C, H, W).astype(NP_DTYPE)
    maps = []
    for k in range(N_CORES):
        blk = flat[k * PLANES_PER_CORE:(k + 1) * PLANES_PER_CORE]
        m = np.ascontiguousarray(blk.transpose(0, 2, 1)).reshape(NCOLS, H)
        a = np.ascontiguousarray(
            m[:NA].reshape(P, QA, H).transpose(0, 2, 1)
        )
        bcols = np.concatenate(
            [m[NA:], np.zeros((NB - (NCOLS - NA), H), NP_DTYPE)], axis=0
        )
        b = np.ascontiguousarray(
            bcols.reshape(NB_P, QB, H).transpose(0, 2, 1)
        ).reshape(NB_P, H // 8, 8, QB)
        maps.append({"xa": a, "xb": b})
    return maps


def assemble_out(results) -> np.ndarray:
    """Per-core {ya, yb} -> fp32 (B,C,H,W)."""
    blocks = []
    for r in results:
        ma = r["ya"].transpose(0, 2, 1).reshape(NA, H)
        mb = (
            r["yb"].reshape(NB_P, H, QB).transpose(0, 2, 1).reshape(NB, H)
        )[: NCOLS - NA]
        m = np.concatenate([ma, mb], axis=0)  # [NCOLS, H]
        blk = m.reshape(PLANES_PER_CORE, W, H).transpose(0, 2, 1)
        blocks.append(blk)
    out = np.concatenate(blocks, axis=0)
    return out.reshape(B, C, H, W).astype(np.float32)


def kernel(x: np.ndarray) -> np.ndarray:
    assert x.shape == (B, C, H, W), x.shape
    in_maps = make_in_maps(x)
    nc = _get_module()
    res = run_bass_kernel_spmd(nc, in_maps, list(range(N_CORES)))
    return assemble_out(res.results)


# revision 10
# speedup vs baseline: 1.8904x; 1.2623x over previous
"""BottomPool (cumulative max along H) Trainium2 Bass kernel.

Full input x: (16, 256, 128, 128) fp32. out[b,c,h,w] = max_{h'<=h} x[b,c,h',w].

Strategy: data-parallel over the 4096 (b,c) planes -> 512 planes per core.
The kernel is HBM-bandwidth-bound (one read + one write of the full tensor
on a shared ~430 GB/s-per-core DMA fabric), so:

- Device I/O is fp16: the host converts fp32 -> fp16 (round-to-nearest,
  max rel quantization error 2^-11 ~ 0.05%, vs the 2e-2 gate), the device
  cummax runs in fp16 (max of rounded values == rounded max: rounding is
  monotone), and the host upcasts the result back to fp32. Halves traffic.
- The host pre-packs each core's 512 planes as [p=128, h, q*w=512] with
  plane = q*128 + p, so a DMA tile [128, seg, 512] has one fully
  contiguous 16KB HBM run per partition (max descriptor efficiency) and
  each DVE row op [128, 512] reads/writes one contiguous 1KB run per
  partition (max DVE rate; the strided [p,q,h,w] layout cost ~2.3x on
  the serial DVE chain, which paces the pipeline tail).

The cummax itself is a serial chain of [128, 512] DVE tensor_max ops
(one per h-row), carried across tiles. No cross-core communication.

Notes from a follow-up optimization session (schedules that did NOT beat
this one on hardware, kept here so they are not retried blindly):
- All-loads-first + in-place chain + FIFO stores (zero-slack bus): 112us.
  SDMA engine 15 intermittently degrades to ~78% throughput (known trn2
  erratum, engines 7/15); with zero bus slack its backlog serializes at
  the end. This baseline's load/store interleaving leaves gaps that
  absorb the erratum, which is why it wins in practice.
- Rebalancing bytes away from engine 15 (descriptors round-robin
  engine = desc_index % 16 per transfer, so 15-partition transfers skip
  E15) is layout-feasible, but 15-partition transfers run at ~7.5 GB/s
  per engine (vs 26.8 for 8-descriptor uniform transfers) and fused
  unequal-width rows produce sub-512B descriptor chunks (~200 GB/s):
  both variants measured 132-198us.
"""

import numpy as np

import concourse.tile as tile
from concourse import bacc, mybir
from concourse.bass_utils import run_bass_kernel_spmd

N_CORES = 8
B, C, H, W = 16, 256, 128, 128
P = 128  # SBUF partitions
PLANES_PER_CORE = (B * C) // N_CORES  # 512
Q = PLANES_PER_CORE // P  # 4 planes stacked along the free dim
QW = Q * W  # 512 fp16 elems = 1KB per partition per h-row
DTYPE = "float16"  # device I/O + compute dtype
NP_DTYPE = np.float16


def build_module(h=H, hs=16, n_cores=N_CORES, bufs_in=4, bufs_out=4,
                 load_engines=("sync",), store_engines=("scalar",),
                 hsegs=None, store_seg=None, dtype=DTYPE, qw=QW,
                 use_stt=False, store_lag=2, first_load_engine=None):
    """Build + compile the per-core Bass module (same program on all cores).

    Per-core I/O is host-packed [P, h, qw] (see module docstring). Tiles
    are [P, seg, qw]; per partition a tile's HBM source is one contiguous
    seg*qw*2-byte run. Loads issue on nc.sync (SP HWDGE ring); stores on
    nc.scalar (ACT ring) so a store blocked on compute doesn't
    head-of-line-block loads.
    """
    if hsegs is None:
        assert h % hs == 0
        hsegs = [hs] * (h // hs)
    assert sum(hsegs) == h, (hsegs, h)
    mdt = getattr(mybir.dt, dtype)
    nc = bacc.Bacc(
        "TRN2", target_bir_lowering=False, debug=False, num_devices=n_cores
    )
    x = nc.dram_tensor("x", [P, h, qw], mdt, kind="ExternalInput").ap()
    y = nc.dram_tensor("y", [P, h, qw], mdt, kind="ExternalOutput").ap()

    with tile.TileContext(nc) as tc:
        load_engs = [getattr(nc, e) for e in load_engines]
        store_engs = [getattr(nc, e) for e in store_engines]
        with (
            tc.tile_pool(name="pin", bufs=bufs_in) as pin,
            tc.tile_pool(name="pout", bufs=bufs_out) as pout,
            tc.tile_pool(name="pgate", bufs=1) as pgate,
        ):
            gate = (
                pgate.tile([P, 1, 1], mdt, name="gate") if store_lag else None
            )
            prev = None
            h0 = 0
            si = 0
            pending = []  # deferred stores: (y_slice, tout_slice)
            for ti, seg in enumerate(hsegs):
                sseg = store_seg or seg
                assert seg % sseg == 0
                tin = pin.tile([P, seg, qw], mdt)
                if ti == 0 and first_load_engine:
                    # The ACT engine clears the entry barriers ~1.3us before
                    # SP, so issuing the first load there starts the bus
                    # earlier; later stores queue behind only this one load.
                    getattr(nc, first_load_engine).dma_start(
                        tin[:], x[:, h0:h0 + seg, :]
                    )
                else:
                    load_engs[ti % len(load_engs)].dma_start(
                        tin[:], x[:, h0:h0 + seg, :]
                    )
                if store_lag and pending and ti >= store_lag:
                    # Gate the next deferred store on THIS tile's load: a
                    # 1-elem copy on the store engine stalls its stream
                    # until load ti lands, keeping loads `store_lag` tiles
                    # ahead of stores in DGE arbitration (loads gate the
                    # whole pipeline; idle bus early beats idle bus late).
                    store_engs[0].activation(
                        gate[:], tin[:, 0:1, 0:1],
                        mybir.ActivationFunctionType.Copy,
                    )
                    dst, src = pending.pop(0)
                    store_engs[0].dma_start(dst, src)
                tout = pout.tile([P, seg, qw], mdt)
                for hh in range(seg):
                    cur = tin[:, hh, :]
                    o = tout[:, hh, :]
                    if prev is None:
                        nc.vector.tensor_copy(o, cur)
                    elif use_stt:
                        nc.vector.scalar_tensor_tensor(
                            o, cur, 0.0, prev,
                            mybir.AluOpType.bypass, mybir.AluOpType.max,
                        )
                    else:
                        nc.vector.tensor_max(o, cur, prev)
                    prev = tout[:, hh, :]
                    if (hh + 1) % sseg == 0:
                        s0 = hh + 1 - sseg
                        dst = y[:, h0 + s0:h0 + hh + 1, :]
                        src = tout[:, s0:hh + 1, :]
                        if store_lag:
                            pending.append((dst, src))
                        else:
                            store_engs[si % len(store_engs)].dma_start(
                                dst, src
                            )
                            si += 1
                h0 += seg
            for dst, src in pending:
                store_engs[0].dma_start(dst, src)
    nc.compile()
    return nc


_NC_CACHE = {}


def _get_module():
    if "nc" not in _NC_CACHE:
        _NC_CACHE["nc"] = build_module()
    return _NC_CACHE["nc"]


def make_in_maps(x: np.ndarray) -> list:
    """fp32 (B,C,H,W) -> per-core fp16 [P, H, QW] packed inputs."""
    flat = np.asarray(x).reshape(B * C, H, W).astype(NP_DTYPE)
    maps = []
    for k in range(N_CORES):
        blk = flat[k * PLANES_PER_CORE:(k + 1) * PLANES_PER_CORE]
        # [Q, P, H, W] -> [P, H, Q, W] -> [P, H, QW]; plane = q*P + p
        packed = np.ascontiguousarray(
            blk.reshape(Q, P, H, W).transpose(1, 2, 0, 3)
        ).reshape(P, H, QW)
        maps.append({"x": packed})
    return maps


def assemble_out(results) -> np.ndarray:
    """Per-core fp16 [P, H, QW] outputs -> fp32 (B,C,H,W)."""
    blocks = []
    for r in results:
        yk = r["y"].reshape(P, H, Q, W).transpose(2, 0, 1, 3)
        blocks.append(yk.reshape(PLANES_PER_CORE, H, W))
    out = np.concatenate(blocks, axis=0)
    return out.reshape(B, C, H, W).astype(np.float32)


def kernel(x: np.ndarray) -> np.ndarray:
    assert x.shape == (B, C, H, W), x.shape
    in_maps = make_in_maps(x)
    nc = _get_module()
    res = run_bass_kernel_spmd(nc, in_maps, list(range(N_CORES)))
    return assemble_out(res.results)
